# revision 5
# baseline (speedup 1.0000x reference)
"""Conv2d 3x3 VALID kernel for Trainium2, batch-sharded across 8 NeuronCores.

Problem: input [32,128,64,64] f32, weights [256,128,3,3] f32 ->
output [32,256,62,62] f32 (stride 1, no padding).

Strategy (per core, 4 images):
  - Cin=128 == SBUF partition dim == matmul contraction dim.
  - Input image b lives in SBUF as [128, 4096] (row-major h*64+w).
  - out[y, x] = sum_{kh,kw,ci} in[ci, (y+kh)*64 + x+kw] * W[co,ci,kh,kw].
    For a block of 8 output rows and tap (kh,kw), the rhs is the strided AP
    in_sb[:, (y0+kh)*64+kw :][8 rows step 64, 62 cols step 1] -> N=496
    moving columns, accumulated over the 9 taps into one PSUM bank.
  - Cout=256 -> two halves of 128 (PSUM partition limit).
  - Weights are DMA'd raw [co,(ci kh kw)] and transposed on-chip with PE
    transposes into lhsT layout [ci, tap*256 + half*128 + co].
  - matmuls run as float32r (fp32 bits, 1 cycle/row at N>=256). The walrus
    birverifier requires every producer feeding an FP32r matmul to emit
    FP32r-typed output, hence the bitcasts on the DMAs/copies.

Perf notes (trace-driven):
  - MM cadence is stream-bound (N/2.4GHz) plus ~23-26ns/MM of fixed
    overhead from the per-instruction tile tick (EVT_SEM write). Only the
    group-final MM's count is ever waited on, so _elide_mm_ticks() zeroes
    the 8 intermediate increments per 9-tap group and compensates on the
    final MM, preserving every cumulative threshold.
  - Head: weights DMA split per Cout-half and image 0 split small-first so
    the first conv group can start as soon as h0 weights + 16 input rows
    land; h1 transposes are deferred until after the h0 conv stream.
"""

import numpy as np

import bass_rust
import concourse.bass as bass
import concourse.mybir as mybir
import concourse.tile as tile
from concourse import bacc
from concourse.bass_utils import run_bass_kernel_spmd
from concourse.masks import make_identity

F32 = mybir.dt.float32
F32R = mybir.dt.float32r

B, CIN, H, W = 32, 128, 64, 64
COUT, KH, KW = 256, 3, 3
OH, OW = H - KH + 1, W - KW + 1  # 62, 62
N_CORES = 8
BL = B // N_CORES  # 4 images per core

IMG_STRIDE = H * W  # 4096
W_FREE = CIN * KH * KW  # 1152
N_TAPS = KH * KW  # 9
ROWS_PER_CHUNK = 8  # 8 output rows x 62 cols = 496 <= 512 (one PSUM bank)


def _conv_body(nc, tc, out_d, x_d, w_d, use_f32r=True):
    mm_dt = F32R if use_f32r else F32
    x_r = x_d.rearrange("b c h w -> b c (h w)")  # [BL, 128, 4096]
    w_r = w_d.rearrange("co ci kh kw -> co (ci kh kw)")  # [256, 1152]

    with (
        tc.tile_pool(name="const", bufs=1) as cpool,
        tc.tile_pool(name="psum", bufs=8, space=bass.MemorySpace.PSUM) as psum_pool,
        tc.tile_pool(name="outp", bufs=4) as out_pool,
    ):
        in_sb = cpool.tile([128, BL * IMG_STRIDE], F32)
        w_raw = cpool.tile([128, 2 * W_FREE], F32)
        w_l = cpool.tile([128, N_TAPS * COUT], F32)  # [ci, t*256 + h*128 + co]
        ident = cpool.tile([128, 128], F32)

        make_identity(nc, ident)

        # Weights first (longest dependency chain: DMA -> transpose -> copy),
        # one dma_start per Cout-half so the h=0 transposes start ~1.7us
        # before the h=1 rows finish landing.
        for h in range(2):
            nc.sync.dma_start(
                out=w_raw[:, h * W_FREE : (h + 1) * W_FREE],
                in_=w_r.rearrange("(h p) c -> h p c", h=2)[h],
            )
        # Image 0 next (needed by the first conv matmuls) in staircase
        # pieces sized so each row-block's rows land before the conv stream
        # reaches it; remaining images whole.
        for b in range(BL):
            pieces = (
                [(0, 1024), (1024, 2048), (2048, 4096)] if b == 0 else [(0, 4096)]
            )
            for c0, c1 in pieces:
                nc.sync.dma_start(
                    out=in_sb[
                        :, b * IMG_STRIDE + c0 : b * IMG_STRIDE + c1
                    ].bitcast(mm_dt),
                    in_=x_r[b][:, c0:c1].bitcast(mm_dt),
                )

        # Transpose weights: w_raw half h viewed as [co, (ci t)] -> per tap
        # [co, ci] (ci at stride 9) -> PE transpose -> [ci, co].
        def transpose_half(h):
            w_v = w_raw[:, h * W_FREE : (h + 1) * W_FREE].rearrange(
                "p (ci t) -> p t ci", t=N_TAPS
            )
            for t in range(N_TAPS):
                ps = psum_pool.tile([128, 512], F32, tag="ps")
                nc.tensor.transpose(ps[:, :128], w_v[:, t, :], ident)
                nc.vector.tensor_copy(
                    w_l[:, t * COUT + h * 128 : t * COUT + h * 128 + 128].bitcast(
                        mm_dt
                    ),
                    ps[:, :128],
                )

        def conv_half(h):
            for b in range(BL):
                img_v = in_sb[
                    :, b * IMG_STRIDE : (b + 1) * IMG_STRIDE
                ].rearrange("p (r x) -> p r x", x=W)  # [128, 64, 64]
                for y0 in range(0, OH, ROWS_PER_CHUNK):
                    nrows = min(ROWS_PER_CHUNK, OH - y0)
                    size = nrows * OW
                    ps = psum_pool.tile([128, 512], F32, tag="ps")
                    ps_v = ps[:, :size].rearrange("p (r x) -> p r x", x=OW)
                    for t in range(N_TAPS):
                        kh, kw = divmod(t, KW)
                        lhsT = w_l[:, t * COUT + h * 128 : t * COUT + h * 128 + 128]
                        # rhs: rectangular window, nrows stride-64 rows x 62 cols
                        rhs = img_v[:, y0 + kh : y0 + kh + nrows, kw : kw + OW]
                        if use_f32r:
                            lhsT = lhsT.bitcast(F32R)
                            rhs = rhs.bitcast(F32R)
                        nc.tensor.matmul(
                            ps_v,
                            lhsT,
                            rhs,
                            start=(t == 0),
                            stop=(t == N_TAPS - 1),
                        )
                    ot = out_pool.tile([128, ROWS_PER_CHUNK * OW], F32)
                    nc.vector.tensor_copy(ot[:, :size], ps[:, :size])
                    nc.sync.dma_start(
                        out=out_d[b, h * 128 : (h + 1) * 128, y0 : y0 + nrows, :],
                        in_=ot[:, :size].rearrange("p (r x) -> p r x", x=OW),
                    )

        # h=1 transposes deferred: the PE runs 9 transposes up front instead
        # of 18, so the first conv group starts earlier; the h=1 weights are
        # only read ~35us into the conv stream.
        transpose_half(0)
        conv_half(0)
        transpose_half(1)
        conv_half(1)


def _elide_mm_ticks(nc):
    """Remove the per-MM tile-tick semaphore increments inside 9-tap matmul
    accumulation groups and renumber every wait threshold on that sem.

    Tile gives every instruction a completion increment on its engine's
    tick semaphore (~23ns of PE time each), but consumers only ever wait
    on the cumulative count reached at a group-final MM. Walrus requires
    UpdateValue == 1, so instead of compensating on the final MM we drop
    the intermediate increments and rewrite all waits into the new (kept
    increments only) numbering. A wait that referenced an elided count is
    bumped to the next kept increment (the group-final MM), which
    completes at-or-after the original trigger point.
    """
    insts_all = [
        i for fn in nc.m.functions for blk in fn.blocks for i in blk.instructions
    ]

    pe_sem_ids = set()
    for i in insts_all:
        if isinstance(i, mybir.InstMatmult) and i.sync_info is not None:
            for u in i.sync_info.on_update:
                if u.update_mode == "sem-inc":
                    pe_sem_ids.add(u.id)
    assert len(pe_sem_ids) == 1, pe_sem_ids
    pe_sem = next(iter(pe_sem_ids))

    def pe_incs(i):
        if i.sync_info is None:
            return []
        return [
            u
            for u in i.sync_info.on_update
            if u.id == pe_sem and u.update_mode == "sem-inc"
        ]

    # every instruction that bumps the PE tick sem, in program order
    incs = [i for i in insts_all if pe_incs(i)]
    for i in incs:
        ups = pe_incs(i)
        assert len(ups) == 1 and ups[0].update_value == 1, (i.name, ups)

    waited = set()
    for i in insts_all:
        si = i.sync_info
        if si is None:
            continue
        for w in si.on_wait:
            if w.id == pe_sem:
                assert w.wait_mode == "sem-ge-imm" and w.wait_reg is None, w
                waited.add(w.wait_value)

    # keep: everything except unwaited mid-group conv matmuls
    kept = []
    for old_cum, i in enumerate(incs, start=1):
        elide = (
            isinstance(i, mybir.InstMatmult)
            and not i.is_transpose
            and not i.stop_tensor_calc
            and old_cum not in waited
        )
        kept.append(not elide)
    # new cumulative numbering (prefix sums of kept)
    newc = []
    c = 0
    for k in kept:
        c += k
        newc.append(c)

    n_elided = 0
    for i, k in zip(incs, kept):
        if not k:
            si = i.sync_info
            i.sync_info = bass_rust.SyncInfo(
                on_wait=list(si.on_wait),
                on_update=[
                    u
                    for u in si.on_update
                    if not (u.id == pe_sem and u.update_mode == "sem-inc")
                ],
            )
            n_elided += 1

    for i in insts_all:
        si = i.sync_info
        if si is None:
            continue
        for w in si.on_wait:
            if w.id == pe_sem:
                v = w.wait_value
                assert 1 <= v <= len(incs), (i.name, v)
                w.wait_value = newc[v - 1] + (0 if kept[v - 1] else 1)
    return n_elided


def build_module(use_f32r=True, elide_ticks=True):
    nc = bacc.Bacc(
        "TRN2", target_bir_lowering=False, debug=False, num_devices=N_CORES
    )
    x_d = nc.dram_tensor(
        "input_image", [BL, CIN, H, W], F32, kind="ExternalInput"
    ).ap()
    w_d = nc.dram_tensor("weights", [COUT, CIN, KH, KW], F32, kind="ExternalInput").ap()
    out_d = nc.dram_tensor("out", [BL, COUT, OH, OW], F32, kind="ExternalOutput").ap()
    with tile.TileContext(nc) as tc:
        _conv_body(nc, tc, out_d, x_d, w_d, use_f32r=use_f32r)
    if elide_ticks:
        _elide_mm_ticks(nc)
    nc.compile()
    return nc


_NC_CACHE = {}


def _get_module(use_f32r=True):
    key = use_f32r
    if key not in _NC_CACHE:
        _NC_CACHE[key] = build_module(use_f32r=use_f32r)
    return _NC_CACHE[key]


def kernel(input_image: np.ndarray, weights: np.ndarray) -> np.ndarray:
    input_image = np.ascontiguousarray(input_image, dtype=np.float32)
    weights = np.ascontiguousarray(weights, dtype=np.float32)
    nc = _get_module()
    in_maps = [
        {
            "input_image": input_image[i * BL : (i + 1) * BL],
            "weights": weights,
        }
        for i in range(N_CORES)
    ]
    res = run_bass_kernel_spmd(nc, in_maps, list(range(N_CORES))).results
    return np.concatenate([r["out"] for r in res], axis=0)
